# revision 15
# baseline (speedup 1.0000x reference)
"""ANFIS forward kernel for Trainium2 (8 NeuronCores, data-parallel over batch).

Problem shapes (hardcoded): B=16384, R=512 rules, F=32 features, O=8 outputs.

Math (identical to the reference, re-associated for the PE):
  a[r]            = -1 / (2*(|w[r]|+0.1)^2)
  v[r,b]          = a[r] * (x2[b] + c2[r] - 2*cross[b,r])      (= -dist/(2 s^2))
  f[r,b]          = exp(v[r,b])                                 (UNnormalized)
  H[b, o*33+f]    = sum_r f[r,b] * Waug[r,f,o]   (f=32 slot holds the bias)
  H[b, 264]       = sum_r f[r,b]                  (= S[b])
  out[b,o]        = (sum_f xaug[b,f]*H[b,o*33+f]) / (S[b]+1e-8)

Device mapping per core (2048 batch rows, 4 groups of 512):
  MM1 (PE):  psum[128r, 512b] = daug[:, rc]^T @ xaug_g, float32r with K=34.
             On this part f32r streams ~1 col/cycle (same as fp16), so plain
             fp32 beats the old fp16 hi/lo 3-pass trick AND avoids the
             ~190ns PE mode switch between fp16 MM1s and f32r MM2s.
  EXP (ACT): firing tile = exp(psum), fp32 (firing spans e^-13..e^-128;
             16-bit floats would flush/degrade columns whose S ~ 1e-8).
  MM2 (PE):  psum_h[128b, 266] += firing[rc][:, j128]^T @ wt[rc] in float32r
             (~124ns issue rate at N=266 once the PE clock boosts).  Software
             pipelined one group behind MM1 so all four exp tiles of a group
             are ready before its MM2 starts.
  Combine, per j-PAIR over a 2-bank psum tile (GPSIMD cannot read PSUM, so
             everything touching psum is DVE): broadcast mul (xaug repeated
             over o via 0-stride AP) into bf16 scratch + reduce_sum +
             eps-add; one reciprocal per group; final scale on GpSimd
             (SBUF-only) to keep it off DVE.

DMA notes (this part's HWDGE):
  * dma_start issue costs ~650-900ns serialized on the issuing queue;
    first packet lands ~1.8us after issue; a transfer's descriptors spread
    over roughly ceil(ndesc/34) of the 16 DMA engines, so small forced
    descriptors (max_dma_last_dim, in BYTES) buy engine parallelism.
  * The scalar-queue (ACT HWDGE) starts packets earlier than the sync queue,
    so the critical transfer (daug + batch group 0) goes there first.
  * A few junk matmuls on a memset tile keep the PE from idling during the
    input DMA (the PE clock ramps ~2x after ~5us of sustained activity).
  * out = [128, NG, JG*O] f32 partition-major (host inverse-permutes after).

Notes:
  * Built on bacc.Bacc + nc.compile(): generate_event_semaphores legalizes the
    1-wait-per-instruction TRN2 limit, so multi-dep matmuls/DMAs are fine.
  * tensor_tensor_reduce hangs this hardware/runtime combination - do not use.
  * The ~6.5us end-of-kernel semaphore-reset chain is fixed bacc epilogue
    (all 256 sems reset serially across engines) - not reducible from here.
"""

import numpy as np

import concourse.bacc as bacc
import concourse.bass as bass
import concourse.tile as tile
from concourse import mybir
from concourse.bass_utils import run_bass_kernel_spmd

B, R, F, O = 16384, 512, 32, 8
NCORES = 8
BL = B // NCORES           # 2048 batch rows per core
GW = 512                   # batch-group width for MM1 streaming
NG = BL // GW              # 4 groups per core
JG = GW // 128             # 4 b128 tiles per group
K1 = F + 2                 # 34 = x rows + ones row + x2 row
F1 = F + 1                 # 33 = x features + ones (bias slot)
NC2 = O * F1 + 2           # 266 = (o,f+bias) block + S + pad (fp32r even dst)
RC = R // 128              # 4 rule chunks
SCOL = O * F1              # 264: column holding S = sum_r firing
XCW = JG * F1              # 132 xc row elems

F32 = mybir.dt.float32
BF16 = mybir.dt.bfloat16
F32R = mybir.dt.float32r
EXP = mybir.ActivationFunctionType.Exp

_PROGRAM = None


def _build_program():
    nc = bacc.Bacc()
    xa_d = nc.declare_dram_parameter("xa", [K1, 2, GW], F32R, isOutput=False)
    xb_d = nc.declare_dram_parameter("xb", [K1, NG - 1, GW], F32R, isOutput=False)
    wt_d = nc.declare_dram_parameter("wt", [128, RC, NC2], F32R, isOutput=False)
    xc_d = nc.declare_dram_parameter("xc", [128, NG, XCW], F32, isOutput=False)
    out_d = nc.declare_dram_parameter("out", [128, NG, JG * O], F32, isOutput=True)

    with tile.TileContext(nc) as tc:
        with (
            tc.tile_pool(name="one", bufs=1) as one,
            tc.tile_pool(name="ft", bufs=10) as ft_pool,
            tc.tile_pool(name="scr", bufs=2) as scr_pool,
            tc.tile_pool(name="sm", bufs=2) as sm_pool,
            tc.tile_pool(name="ot", bufs=2) as ot_pool,
            tc.tile_pool(name="ps1", bufs=2, space="PSUM") as ps1_pool,
            tc.tile_pool(name="psh", bufs=3, space="PSUM") as psh_pool,
        ):
            # PE warmup while the input DMA is in flight.
            warm = one.tile([128, GW], F32, tag="warm")
            nc.gpsimd.memset(warm[:], 0)
            # slot 0 = daug, slots 1..4 = batch groups.
            xd_sb = one.tile([K1, NG + 1, GW], F32R, tag="xd")
            nc.scalar.dma_start(
                out=xd_sb[:, 0:2], in_=xa_d[:], max_dma_last_dim=512
            )
            w_sb = one.tile([128, RC, NC2], F32R, tag="wt")
            nc.sync.dma_start(out=w_sb[:], in_=wt_d[:], max_dma_last_dim=1064)
            nc.scalar.dma_start(
                out=xd_sb[:, 2:], in_=xb_d[:], max_dma_last_dim=1024
            )
            xc_sb = one.tile([128, NG, JG, F1], F32, tag="xc")
            nc.scalar.dma_start(
                out=xc_sb[:],
                in_=xc_d[:].rearrange("p g (j f) -> p g j f", f=F1),
                max_dma_last_dim=528,
            )
            wps = ps1_pool.tile([128, GW], F32, tag="ps1")
            for _ in range(3):
                nc.tensor.matmul(
                    wps[:],
                    warm[:, 0:128].bitcast(F32R),
                    warm[:].bitcast(F32R),
                    start=True,
                    stop=True,
                )

            fts = [[None] * RC for _ in range(NG)]

            def emit_pair(g, jj, osum_g, sden_g):
                """MM2 + numerator/denominator for j-chunks (2jj, 2jj+1)."""
                psh = psh_pool.tile([128, 2, GW], F32, tag="psh")
                for rc in range(RC):
                    for jp in range(2):
                        j = 2 * jj + jp
                        nc.tensor.matmul(
                            psh[:, jp, 0:NC2],
                            fts[g][rc][:, j * 128:(j + 1) * 128],
                            w_sb[:, rc, :],
                            start=(rc == 0),
                            stop=(rc == RC - 1),
                        )
                # GPSIMD cannot read PSUM, so everything touching psh is DVE.
                # bf16 scratch halves the reduce's input cost.
                xv = xc_sb[:, g, 2 * jj:2 * jj + 2, :]     # [128, 2, F1]
                xb = bass.AP(
                    tensor=xv.tensor,
                    offset=xv.offset,
                    ap=[xv.ap[0], xv.ap[1], [0, O], xv.ap[2]],
                )
                scratch = scr_pool.tile([128, 2, O, F1], BF16, tag="scr")
                nc.vector.tensor_mul(
                    scratch[:],
                    psh[:, :, 0:SCOL].rearrange("p j (o f) -> p j o f", o=O),
                    xb,
                )
                nc.vector.reduce_sum(
                    out=osum_g[:, jj], in_=scratch[:], axis=mybir.AxisListType.X
                )
                nc.vector.tensor_scalar_add(
                    sden_g[:, jj], psh[:, :, SCOL:SCOL + 1], 1e-8
                )

            def emit_mm2_combine(g):
                out_t = ot_pool.tile([128, JG, O], F32, tag="outt")
                osum_g = sm_pool.tile([128, 2, 2, O], F32, tag="osum")
                sden_g = sm_pool.tile([128, 2, 2, 1], F32, tag="sden")
                emit_pair(g, 0, osum_g, sden_g)
                emit_pair(g, 1, osum_g, sden_g)
                rec = sm_pool.tile([128, 2, 2, 1], F32, tag="rec")
                nc.vector.reciprocal(rec[:], sden_g[:])
                ra = rec[:]
                rb = bass.AP(
                    tensor=ra.tensor,
                    offset=ra.offset,
                    ap=[ra.ap[0], ra.ap[1], ra.ap[2], [0, O]],
                )
                # Final scale is SBUF-only, so GpSimd takes it off DVE.
                nc.gpsimd.tensor_mul(
                    out_t[:].rearrange("p (h t) o -> p h t o", h=2), osum_g[:], rb
                )
                nc.sync.dma_start(out=out_d[:, g], in_=out_t[:])

            for g in range(NG):
                for rc in range(RC):
                    ps1 = ps1_pool.tile([128, GW], F32, tag="ps1")
                    nc.tensor.matmul(
                        ps1[:],
                        xd_sb[:, 0, rc * 128:(rc + 1) * 128],
                        xd_sb[:, 1 + g, :],
                        start=True,
                        stop=True,
                    )
                    ft = ft_pool.tile([128, GW], F32R, tag="ft")
                    nc.scalar.activation(ft[:], ps1[:], EXP)
                    fts[g][rc] = ft
                # MM2 one group behind: its 4 exp tiles are already done.
                if g > 0:
                    emit_mm2_combine(g - 1)
            emit_mm2_combine(NG - 1)
    nc.compile()
    return nc


def get_program():
    global _PROGRAM
    if _PROGRAM is None:
        _PROGRAM = _build_program()
    return _PROGRAM


def make_in_maps(inputs, centers, widths, consequent_w, consequent_b):
    x = np.ascontiguousarray(np.asarray(inputs, dtype=np.float32))
    c64 = np.asarray(centers, dtype=np.float64)
    w64 = np.asarray(widths, dtype=np.float64)
    W = np.asarray(consequent_w, dtype=np.float32)
    cb = np.asarray(consequent_b, dtype=np.float32)

    s = np.abs(w64) + 0.1
    a = -1.0 / (2.0 * s * s)                       # [R]

    daug = np.empty((K1, R), dtype=np.float64)
    daug[:F] = (c64 * (-2.0 * a)[:, None]).T       # -2*c[r,f]*a[r]
    daug[F] = (c64 * c64).sum(axis=1) * a          # c2[r]*a[r]  (pairs with ones)
    daug[F + 1] = a                                # a[r]        (pairs with x2)
    daug = daug.astype(np.float32)

    # Waug columns: col o*33+f = W[r,f,o] (f<32), col o*33+32 = cb[r,o],
    # col 264 = 1.0 (S), col 265 = 0 (pad).
    wt = np.zeros((R, NC2), dtype=np.float32)
    for o in range(O):
        wt[:, o * F1:o * F1 + F] = W[:, :, o]
        wt[:, o * F1 + F] = cb[:, o]
    wt[:, SCOL] = 1.0
    wt = np.ascontiguousarray(wt.reshape(RC, 128, NC2).transpose(1, 0, 2))

    x2 = np.einsum("bf,bf->b", x, x).astype(np.float32)
    xat = np.empty((K1, B), dtype=np.float32)
    xat[:F] = x.T
    xat[F] = 1.0
    xat[F + 1] = x2

    xaug = np.empty((B, F1), dtype=np.float32)
    xaug[:, :F] = x
    xaug[:, F] = 1.0

    in_maps = []
    for ci in range(NCORES):
        sl = slice(ci * BL, (ci + 1) * BL)
        xd = np.empty((K1, NG + 1, GW), dtype=np.float32)
        xd[:, 0, :] = daug
        xd[:, 1:, :] = xat[:, sl].reshape(K1, NG, GW)
        xc = (
            xaug[sl].reshape(NG, JG, 128, F1)
            .transpose(2, 0, 1, 3)
            .reshape(128, NG, XCW)
        )
        in_maps.append({
            "xa": np.ascontiguousarray(xd[:, 0:2]),
            "xb": np.ascontiguousarray(xd[:, 2:]),
            "wt": wt,
            "xc": np.ascontiguousarray(xc),
        })
    return in_maps


def assemble_out(results):
    """[128, NG, JG*O] per core -> full [B, O] (b = g*512 + j*128 + p)."""
    outs = []
    for r in results:
        o = np.asarray(r["out"]).reshape(128, NG, JG, O)
        outs.append(o.transpose(1, 2, 0, 3).reshape(BL, O))
    return np.concatenate(outs, axis=0)


def _axon_reset():
    """Recover a wedged NeuronCore (NRT_EXEC_UNIT_UNRECOVERABLE) via the axon
    client's reset entry point.  Best-effort: silently skipped off-axon."""
    try:
        import ctypes
        import time

        import jax

        jax.devices()
        lib = ctypes.CDLL("/opt/axon/libaxon_pjrt.so")
        lib.axon_reset.restype = ctypes.c_int64
        lib.axon_reset()
        time.sleep(2)
    except Exception:
        pass


def kernel(inputs, centers, widths, consequent_w, consequent_b):
    nc = get_program()
    in_maps = make_in_maps(inputs, centers, widths, consequent_w, consequent_b)
    last_err = None
    for attempt in range(3):
        try:
            res = run_bass_kernel_spmd(nc, in_maps, list(range(NCORES))).results
            return assemble_out(res)
        except Exception as e:  # wedged device -> reset + retry
            last_err = e
            _axon_reset()
    raise last_err


# revision 16
# speedup vs baseline: 1.5199x; 1.5199x over previous
"""ANFIS forward kernel for Trainium2 (8 NeuronCores, data-parallel over batch).

Problem shapes (hardcoded): B=16384, R=512 rules, F=32 features, O=8 outputs.

Math (identical to the reference, re-associated for the PE):
  a[r]            = -1 / (2*(|w[r]|+0.1)^2)
  v[r,b]          = a[r] * (x2[b] + c2[r] - 2*cross[b,r])      (= -dist/(2 s^2))
  f[r,b]          = exp(v[r,b])                                 (UNnormalized)
  H[b, o*33+f]    = sum_r f[r,b] * Waug[r,f,o]   (f=32 slot holds the bias)
  H[b, 264]       = sum_r f[r,b]                  (= S[b])
  out[b,o]        = (sum_f xaug[b,f]*H[b,o*33+f]) / (S[b]+1e-8)

Device mapping per core (2048 batch rows, 4 groups of 512):
  MM1 (PE):  psum[128r, 512b] = daug[:, rc]^T @ xaug_g, float32r with K=34.
             On this part f32r streams ~1 col/cycle (same as fp16), so plain
             fp32 beats the old fp16 hi/lo 3-pass trick AND avoids the
             ~190ns PE mode switch between fp16 MM1s and f32r MM2s.
  EXP (ACT): firing tile = exp(psum), fp32 (firing spans e^-13..e^-128;
             16-bit floats would flush/degrade columns whose S ~ 1e-8).
  MM2 (PE):  psum_h[128b, 266] += firing[rc][:, j128]^T @ wt[rc] in float32r
             (~124ns issue rate at N=266 once the PE clock boosts).  Software
             pipelined one group behind MM1 so all four exp tiles of a group
             are ready before its MM2 starts.
  Combine, per j-PAIR over a 2-bank psum tile (GPSIMD cannot read PSUM, so
             everything touching psum is DVE): broadcast mul (xaug repeated
             over o via 0-stride AP) into bf16 scratch + reduce_sum +
             eps-add; one reciprocal per group; final scale on GpSimd
             (SBUF-only) to keep it off DVE.

DMA notes (this part's HWDGE):
  * dma_start issue costs ~650-900ns serialized on the issuing queue;
    first packet lands ~1.8us after issue; a transfer's descriptors spread
    over roughly ceil(ndesc/34) of the 16 DMA engines, so small forced
    descriptors (max_dma_last_dim, in BYTES) buy engine parallelism.
  * The scalar-queue (ACT HWDGE) starts packets earlier than the sync queue,
    so the critical transfer (daug + batch group 0) goes there first.
  * A few junk matmuls on a memset tile keep the PE from idling during the
    input DMA (the PE clock ramps ~2x after ~5us of sustained activity).
  * out = [128, NG, JG*O] f32 partition-major (host inverse-permutes after).

Notes:
  * Built on bacc.Bacc + nc.compile(): generate_event_semaphores legalizes the
    1-wait-per-instruction TRN2 limit, so multi-dep matmuls/DMAs are fine.
  * tensor_tensor_reduce hangs this hardware/runtime combination - do not use.
  * The ~6.5us end-of-kernel semaphore-reset chain is fixed bacc epilogue
    (all 256 sems reset serially across engines) - not reducible from here.
"""

import numpy as np

import concourse.bacc as bacc
import concourse.bass as bass
import concourse.tile as tile
from concourse import mybir
from concourse.bass_utils import run_bass_kernel_spmd

B, R, F, O = 16384, 512, 32, 8
NCORES = 8
BL = B // NCORES           # 2048 batch rows per core
GW = 512                   # batch-group width for MM1 streaming
NG = BL // GW              # 4 groups per core
JG = GW // 128             # 4 b128 tiles per group
K1 = F + 2                 # 34 = x rows + ones row + x2 row
KP = 3 * K1                # 102 = packed contraction dim [h;l;h]
F1 = F + 1                 # 33 = x features + ones (bias slot)
NC2 = O * F1 + 2           # 266 = (o,f+bias) block + S + pad (fp32r even dst)
RC = R // 128              # 4 rule chunks
SCOL = O * F1              # 264: column holding S = sum_r firing
XCW = JG * F1              # 132 xc row elems

F32 = mybir.dt.float32
F16 = mybir.dt.float16
BF16 = mybir.dt.bfloat16
F32R = mybir.dt.float32r
EXP = mybir.ActivationFunctionType.Exp

_PROGRAM = None


def _build_program():
    nc = bacc.Bacc()
    xa_d = nc.declare_dram_parameter("xa", [KP, 2, GW], F16, isOutput=False)
    xb_d = nc.declare_dram_parameter("xb", [KP, NG - 1, GW], F16, isOutput=False)
    wt_d = nc.declare_dram_parameter("wt", [128, RC, NC2], F32R, isOutput=False)
    xc_d = nc.declare_dram_parameter("xc", [128, NG, XCW], F32, isOutput=False)
    out_d = nc.declare_dram_parameter("out", [128, NG, JG * O], F32, isOutput=True)

    with tile.TileContext(nc) as tc:
        with (
            tc.tile_pool(name="one", bufs=1) as one,
            tc.tile_pool(name="ft", bufs=10) as ft_pool,
            tc.tile_pool(name="scr", bufs=2) as scr_pool,
            tc.tile_pool(name="sm", bufs=2) as sm_pool,
            tc.tile_pool(name="ot", bufs=2) as ot_pool,
            tc.tile_pool(name="ps1", bufs=2, space="PSUM") as ps1_pool,
            tc.tile_pool(name="psh", bufs=3, space="PSUM") as psh_pool,
        ):
            # PE warmup while the input DMA is in flight.
            warm = one.tile([128, GW], F16, tag="warm")
            nc.gpsimd.memset(warm[:], 0)
            # slot 0 = dpack, slots 1..4 = batch groups.
            xd_sb = one.tile([KP, NG + 1, GW], F16, tag="xd")
            nc.scalar.dma_start(
                out=xd_sb[:, 0:2], in_=xa_d[:], max_dma_last_dim=512
            )
            w_sb = one.tile([128, RC, NC2], F32R, tag="wt")
            nc.sync.dma_start(out=w_sb[:], in_=wt_d[:], max_dma_last_dim=1064)
            nc.scalar.dma_start(
                out=xd_sb[:, 2:], in_=xb_d[:], max_dma_last_dim=1024
            )
            xc_sb = one.tile([128, NG, JG, F1], F32, tag="xc")
            nc.scalar.dma_start(
                out=xc_sb[:],
                in_=xc_d[:].rearrange("p g (j f) -> p g j f", f=F1),
                max_dma_last_dim=528,
            )
            wps = ps1_pool.tile([128, GW], F32, tag="ps1")
            for _ in range(3):
                nc.tensor.matmul(
                    wps[:], warm[:, 0:128], warm[:], start=True, stop=True
                )

            fts = [[None] * RC for _ in range(NG)]

            def emit_pair(g, jj, osum_g, sden_g):
                """MM2 + numerator/denominator for j-chunks (2jj, 2jj+1)."""
                psh = psh_pool.tile([128, 2, GW], F32, tag="psh")
                for rc in range(RC):
                    for jp in range(2):
                        j = 2 * jj + jp
                        nc.tensor.matmul(
                            psh[:, jp, 0:NC2],
                            fts[g][rc][:, j * 128:(j + 1) * 128],
                            w_sb[:, rc, :],
                            start=(rc == 0),
                            stop=(rc == RC - 1),
                        )
                # GPSIMD cannot read PSUM, so everything touching psh is DVE.
                # bf16 scratch halves the reduce's input cost.
                xv = xc_sb[:, g, 2 * jj:2 * jj + 2, :]     # [128, 2, F1]
                xb = bass.AP(
                    tensor=xv.tensor,
                    offset=xv.offset,
                    ap=[xv.ap[0], xv.ap[1], [0, O], xv.ap[2]],
                )
                scratch = scr_pool.tile([128, 2, O, F1], BF16, tag="scr")
                nc.vector.tensor_mul(
                    scratch[:],
                    psh[:, :, 0:SCOL].rearrange("p j (o f) -> p j o f", o=O),
                    xb,
                )
                nc.vector.reduce_sum(
                    out=osum_g[:, jj], in_=scratch[:], axis=mybir.AxisListType.X
                )
                nc.vector.tensor_scalar_add(
                    sden_g[:, jj], psh[:, :, SCOL:SCOL + 1], 1e-8
                )

            def emit_mm2_combine(g):
                out_t = ot_pool.tile([128, JG, O], F32, tag="outt")
                osum_g = sm_pool.tile([128, 2, 2, O], F32, tag="osum")
                sden_g = sm_pool.tile([128, 2, 2, 1], F32, tag="sden")
                emit_pair(g, 0, osum_g, sden_g)
                emit_pair(g, 1, osum_g, sden_g)
                rec = sm_pool.tile([128, 2, 2, 1], F32, tag="rec")
                nc.vector.reciprocal(rec[:], sden_g[:])
                ra = rec[:]
                rb = bass.AP(
                    tensor=ra.tensor,
                    offset=ra.offset,
                    ap=[ra.ap[0], ra.ap[1], ra.ap[2], [0, O]],
                )
                # Final scale is SBUF-only, so GpSimd takes it off DVE.
                nc.gpsimd.tensor_mul(
                    out_t[:].rearrange("p (h t) o -> p h t o", h=2), osum_g[:], rb
                )
                nc.sync.dma_start(out=out_d[:, g], in_=out_t[:])

            for g in range(NG):
                for rc in range(RC):
                    ps1 = ps1_pool.tile([128, GW], F32, tag="ps1")
                    nc.tensor.matmul(
                        ps1[:],
                        xd_sb[:, 0, rc * 128:(rc + 1) * 128],
                        xd_sb[:, 1 + g, :],
                        start=True,
                        stop=True,
                    )
                    ft = ft_pool.tile([128, GW], F32R, tag="ft")
                    nc.scalar.activation(ft[:], ps1[:], EXP)
                    fts[g][rc] = ft
                # MM2 one group behind: its 4 exp tiles are already done.
                if g > 0:
                    emit_mm2_combine(g - 1)
            emit_mm2_combine(NG - 1)
    nc.compile()
    return nc


def get_program():
    global _PROGRAM
    if _PROGRAM is None:
        _PROGRAM = _build_program()
    return _PROGRAM


def _split_f16(arr):
    hi = arr.astype(np.float16)
    lo = (arr - hi.astype(np.float32)).astype(np.float16)
    return np.ascontiguousarray(hi), np.ascontiguousarray(lo)


def make_in_maps(inputs, centers, widths, consequent_w, consequent_b):
    x = np.ascontiguousarray(np.asarray(inputs, dtype=np.float32))
    c64 = np.asarray(centers, dtype=np.float64)
    w64 = np.asarray(widths, dtype=np.float64)
    W = np.asarray(consequent_w, dtype=np.float32)
    cb = np.asarray(consequent_b, dtype=np.float32)

    s = np.abs(w64) + 0.1
    a = -1.0 / (2.0 * s * s)                       # [R]

    daug = np.empty((K1, R), dtype=np.float64)
    daug[:F] = (c64 * (-2.0 * a)[:, None]).T       # -2*c[r,f]*a[r]
    daug[F] = (c64 * c64).sum(axis=1) * a          # c2[r]*a[r]  (pairs with ones)
    daug[F + 1] = a                                # a[r]        (pairs with x2)
    dh, dl = _split_f16(daug.astype(np.float32))
    dpack = np.concatenate([dh, dh, dl], axis=0)   # [102, R]

    # Waug columns: col o*33+f = W[r,f,o] (f<32), col o*33+32 = cb[r,o],
    # col 264 = 1.0 (S), col 265 = 0 (pad).
    wt = np.zeros((R, NC2), dtype=np.float32)
    for o in range(O):
        wt[:, o * F1:o * F1 + F] = W[:, :, o]
        wt[:, o * F1 + F] = cb[:, o]
    wt[:, SCOL] = 1.0
    wt = np.ascontiguousarray(wt.reshape(RC, 128, NC2).transpose(1, 0, 2))

    x2 = np.einsum("bf,bf->b", x, x).astype(np.float32)
    xat = np.empty((K1, B), dtype=np.float32)
    xat[:F] = x.T
    xat[F] = 1.0
    xat[F + 1] = x2
    xah, xal = _split_f16(xat)
    xpk = np.concatenate([xah, xal, xah], axis=0)  # [102, B]

    xaug = np.empty((B, F1), dtype=np.float32)
    xaug[:, :F] = x
    xaug[:, F] = 1.0

    in_maps = []
    for ci in range(NCORES):
        sl = slice(ci * BL, (ci + 1) * BL)
        xd = np.empty((KP, NG + 1, GW), dtype=np.float16)
        xd[:, 0, :] = dpack
        xd[:, 1:, :] = xpk[:, sl].reshape(KP, NG, GW)
        xc = (
            xaug[sl].reshape(NG, JG, 128, F1)
            .transpose(2, 0, 1, 3)
            .reshape(128, NG, XCW)
        )
        in_maps.append({
            "xa": np.ascontiguousarray(xd[:, 0:2]),
            "xb": np.ascontiguousarray(xd[:, 2:]),
            "wt": wt,
            "xc": np.ascontiguousarray(xc),
        })
    return in_maps


def assemble_out(results):
    """[128, NG, JG*O] per core -> full [B, O] (b = g*512 + j*128 + p)."""
    outs = []
    for r in results:
        o = np.asarray(r["out"]).reshape(128, NG, JG, O)
        outs.append(o.transpose(1, 2, 0, 3).reshape(BL, O))
    return np.concatenate(outs, axis=0)


def _axon_reset():
    """Recover a wedged NeuronCore (NRT_EXEC_UNIT_UNRECOVERABLE) via the axon
    client's reset entry point.  Best-effort: silently skipped off-axon."""
    try:
        import ctypes
        import time

        import jax

        jax.devices()
        lib = ctypes.CDLL("/opt/axon/libaxon_pjrt.so")
        lib.axon_reset.restype = ctypes.c_int64
        lib.axon_reset()
        time.sleep(2)
    except Exception:
        pass


def kernel(inputs, centers, widths, consequent_w, consequent_b):
    nc = get_program()
    in_maps = make_in_maps(inputs, centers, widths, consequent_w, consequent_b)
    last_err = None
    for attempt in range(3):
        try:
            res = run_bass_kernel_spmd(nc, in_maps, list(range(NCORES))).results
            return assemble_out(res)
        except Exception as e:  # wedged device -> reset + retry
            last_err = e
            _axon_reset()
    raise last_err


# revision 17
# speedup vs baseline: 1.5248x; 1.0032x over previous
"""ANFIS forward kernel for Trainium2 (8 NeuronCores, data-parallel over batch).

Problem shapes (hardcoded): B=16384, R=512 rules, F=32 features, O=8 outputs.

Math (identical to the reference, re-associated for the PE):
  a[r]            = -1 / (2*(|w[r]|+0.1)^2)
  v[r,b]          = a[r] * (x2[b] + c2[r] - 2*cross[b,r])      (= -dist/(2 s^2))
  f[r,b]          = exp(v[r,b])                                 (UNnormalized)
  H[b, o*33+f]    = sum_r f[r,b] * Waug[r,f,o]   (f=32 slot holds the bias)
  H[b, 264]       = sum_r f[r,b]                  (= S[b])
  out[b,o]        = (sum_f xaug[b,f]*H[b,o*33+f]) / (S[b]+1e-8)

Device mapping per core (2048 batch rows, 4 groups of 512):
  MM1 (PE):  psum[128r, 512b] = daug[:, rc]^T @ xaug_g, float32r with K=34.
             On this part f32r streams ~1 col/cycle (same as fp16), so plain
             fp32 beats the old fp16 hi/lo 3-pass trick AND avoids the
             ~190ns PE mode switch between fp16 MM1s and f32r MM2s.
  EXP (ACT): firing tile = exp(psum), fp32 (firing spans e^-13..e^-128;
             16-bit floats would flush/degrade columns whose S ~ 1e-8).
  MM2 (PE):  psum_h[128b, 266] += firing[rc][:, j128]^T @ wt[rc] in float32r
             (~124ns issue rate at N=266 once the PE clock boosts).  Software
             pipelined one group behind MM1 so all four exp tiles of a group
             are ready before its MM2 starts.
  Combine, per j-PAIR over a 2-bank psum tile (GPSIMD cannot read PSUM, so
             everything touching psum is DVE): broadcast mul (xaug repeated
             over o via 0-stride AP) into bf16 scratch + reduce_sum +
             eps-add; one reciprocal per group; final scale on GpSimd
             (SBUF-only) to keep it off DVE.

DMA notes (this part's HWDGE):
  * dma_start issue costs ~650-900ns serialized on the issuing queue;
    first packet lands ~1.8us after issue; a transfer's descriptors spread
    over roughly ceil(ndesc/34) of the 16 DMA engines, so small forced
    descriptors (max_dma_last_dim, in BYTES) buy engine parallelism.
  * The scalar-queue (ACT HWDGE) starts packets earlier than the sync queue,
    so the critical transfer (daug + batch group 0) goes there first.
  * A few junk matmuls on a memset tile keep the PE from idling during the
    input DMA (the PE clock ramps ~2x after ~5us of sustained activity).
  * out = [128, NG, JG*O] f32 partition-major (host inverse-permutes after).

Notes:
  * Built on bacc.Bacc + nc.compile(): generate_event_semaphores legalizes the
    1-wait-per-instruction TRN2 limit, so multi-dep matmuls/DMAs are fine.
  * tensor_tensor_reduce hangs this hardware/runtime combination - do not use.
  * The ~6.5us end-of-kernel semaphore-reset chain is fixed bacc epilogue
    (all 256 sems reset serially across engines) - not reducible from here.
"""

import numpy as np

import concourse.bacc as bacc
import concourse.bass as bass
import concourse.tile as tile
from concourse import mybir
from concourse.bass_utils import run_bass_kernel_spmd

B, R, F, O = 16384, 512, 32, 8
NCORES = 8
BL = B // NCORES           # 2048 batch rows per core
GW = 512                   # batch-group width for MM1 streaming
NG = BL // GW              # 4 groups per core
JG = GW // 128             # 4 b128 tiles per group
K1 = F + 2                 # 34 = x rows + ones row + x2 row
KP = 3 * K1                # 102 = packed contraction dim [h;l;h]
F1 = F + 1                 # 33 = x features + ones (bias slot)
NC2 = O * F1 + 2           # 266 = (o,f+bias) block + S + pad (fp32r even dst)
RC = R // 128              # 4 rule chunks
SCOL = O * F1              # 264: column holding S = sum_r firing
XCW = JG * F1              # 132 xc row elems

F32 = mybir.dt.float32
F16 = mybir.dt.float16
BF16 = mybir.dt.bfloat16
F32R = mybir.dt.float32r
EXP = mybir.ActivationFunctionType.Exp

_PROGRAM = None


def _build_program():
    nc = bacc.Bacc()
    xa_d = nc.declare_dram_parameter("xa", [KP, 2, GW], F16, isOutput=False)
    xb_d = nc.declare_dram_parameter("xb", [KP, NG - 1, GW], F16, isOutput=False)
    wt_d = nc.declare_dram_parameter("wt", [128, RC, NC2], F32R, isOutput=False)
    xc_d = nc.declare_dram_parameter("xc", [128, NG, XCW], F32, isOutput=False)
    out_d = nc.declare_dram_parameter("out", [128, NG, JG * O], F32, isOutput=True)

    with tile.TileContext(nc) as tc:
        with (
            tc.tile_pool(name="one", bufs=1) as one,
            tc.tile_pool(name="ft", bufs=10) as ft_pool,
            tc.tile_pool(name="scr", bufs=2) as scr_pool,
            tc.tile_pool(name="sm", bufs=2) as sm_pool,
            tc.tile_pool(name="ot", bufs=2) as ot_pool,
            tc.tile_pool(name="ps1", bufs=2, space="PSUM") as ps1_pool,
            tc.tile_pool(name="psh", bufs=3, space="PSUM") as psh_pool,
        ):
            # PE warmup while the input DMA is in flight.
            warm = one.tile([128, GW], F16, tag="warm")
            nc.gpsimd.memset(warm[:], 0)
            # slot 0 = dpack, slots 1..4 = batch groups.
            xd_sb = one.tile([KP, NG + 1, GW], F16, tag="xd")
            nc.scalar.dma_start(
                out=xd_sb[:, 0:2], in_=xa_d[:], max_dma_last_dim=512
            )
            w_sb = one.tile([128, RC, NC2], F32R, tag="wt")
            nc.sync.dma_start(out=w_sb[:], in_=wt_d[:], max_dma_last_dim=1064)
            nc.scalar.dma_start(
                out=xd_sb[:, 2:], in_=xb_d[:], max_dma_last_dim=1024
            )
            xc_sb = one.tile([128, NG, JG, F1], F32, tag="xc")
            nc.scalar.dma_start(
                out=xc_sb[:],
                in_=xc_d[:].rearrange("p g (j f) -> p g j f", f=F1),
                max_dma_last_dim=528,
            )
            wps = ps1_pool.tile([128, GW], F32, tag="ps1")
            for _ in range(8):
                nc.tensor.matmul(
                    wps[:], warm[:, 0:128], warm[:], start=True, stop=True
                )

            fts = [[None] * RC for _ in range(NG)]

            def emit_pair(g, jj, osum_g, sden_g):
                """MM2 + numerator/denominator for j-chunks (2jj, 2jj+1)."""
                psh = psh_pool.tile([128, 2, GW], F32, tag="psh")
                for rc in range(RC):
                    for jp in range(2):
                        j = 2 * jj + jp
                        nc.tensor.matmul(
                            psh[:, jp, 0:NC2],
                            fts[g][rc][:, j * 128:(j + 1) * 128],
                            w_sb[:, rc, :],
                            start=(rc == 0),
                            stop=(rc == RC - 1),
                        )
                # GPSIMD cannot read PSUM, so everything touching psh is DVE.
                # bf16 scratch halves the reduce's input cost.
                xv = xc_sb[:, g, 2 * jj:2 * jj + 2, :]     # [128, 2, F1]
                xb = bass.AP(
                    tensor=xv.tensor,
                    offset=xv.offset,
                    ap=[xv.ap[0], xv.ap[1], [0, O], xv.ap[2]],
                )
                scratch = scr_pool.tile([128, 2, O, F1], BF16, tag="scr")
                nc.vector.tensor_mul(
                    scratch[:],
                    psh[:, :, 0:SCOL].rearrange("p j (o f) -> p j o f", o=O),
                    xb,
                )
                nc.vector.reduce_sum(
                    out=osum_g[:, jj], in_=scratch[:], axis=mybir.AxisListType.X
                )
                nc.vector.tensor_scalar_add(
                    sden_g[:, jj], psh[:, :, SCOL:SCOL + 1], 1e-8
                )

            def emit_mm2_combine(g):
                out_t = ot_pool.tile([128, JG, O], F32, tag="outt")
                osum_g = sm_pool.tile([128, 2, 2, O], F32, tag="osum")
                sden_g = sm_pool.tile([128, 2, 2, 1], F32, tag="sden")
                emit_pair(g, 0, osum_g, sden_g)
                emit_pair(g, 1, osum_g, sden_g)
                rec = sm_pool.tile([128, 2, 2, 1], F32, tag="rec")
                nc.vector.reciprocal(rec[:], sden_g[:])
                ra = rec[:]
                rb = bass.AP(
                    tensor=ra.tensor,
                    offset=ra.offset,
                    ap=[ra.ap[0], ra.ap[1], ra.ap[2], [0, O]],
                )
                # Final scale is SBUF-only, so GpSimd takes it off DVE.
                nc.gpsimd.tensor_mul(
                    out_t[:].rearrange("p (h t) o -> p h t o", h=2), osum_g[:], rb
                )
                nc.sync.dma_start(out=out_d[:, g], in_=out_t[:])

            for g in range(NG):
                for rc in range(RC):
                    ps1 = ps1_pool.tile([128, GW], F32, tag="ps1")
                    nc.tensor.matmul(
                        ps1[:],
                        xd_sb[:, 0, rc * 128:(rc + 1) * 128],
                        xd_sb[:, 1 + g, :],
                        start=True,
                        stop=True,
                    )
                    ft = ft_pool.tile([128, GW], F32R, tag="ft")
                    nc.scalar.activation(ft[:], ps1[:], EXP)
                    fts[g][rc] = ft
                # MM2 one group behind: its 4 exp tiles are already done.
                if g > 0:
                    emit_mm2_combine(g - 1)
            emit_mm2_combine(NG - 1)
    nc.compile()
    return nc


def get_program():
    global _PROGRAM
    if _PROGRAM is None:
        _PROGRAM = _build_program()
    return _PROGRAM


def _split_f16(arr):
    hi = arr.astype(np.float16)
    lo = (arr - hi.astype(np.float32)).astype(np.float16)
    return np.ascontiguousarray(hi), np.ascontiguousarray(lo)


def make_in_maps(inputs, centers, widths, consequent_w, consequent_b):
    x = np.ascontiguousarray(np.asarray(inputs, dtype=np.float32))
    c64 = np.asarray(centers, dtype=np.float64)
    w64 = np.asarray(widths, dtype=np.float64)
    W = np.asarray(consequent_w, dtype=np.float32)
    cb = np.asarray(consequent_b, dtype=np.float32)

    s = np.abs(w64) + 0.1
    a = -1.0 / (2.0 * s * s)                       # [R]

    daug = np.empty((K1, R), dtype=np.float64)
    daug[:F] = (c64 * (-2.0 * a)[:, None]).T       # -2*c[r,f]*a[r]
    daug[F] = (c64 * c64).sum(axis=1) * a          # c2[r]*a[r]  (pairs with ones)
    daug[F + 1] = a                                # a[r]        (pairs with x2)
    dh, dl = _split_f16(daug.astype(np.float32))
    dpack = np.concatenate([dh, dh, dl], axis=0)   # [102, R]

    # Waug columns: col o*33+f = W[r,f,o] (f<32), col o*33+32 = cb[r,o],
    # col 264 = 1.0 (S), col 265 = 0 (pad).
    wt = np.zeros((R, NC2), dtype=np.float32)
    for o in range(O):
        wt[:, o * F1:o * F1 + F] = W[:, :, o]
        wt[:, o * F1 + F] = cb[:, o]
    wt[:, SCOL] = 1.0
    wt = np.ascontiguousarray(wt.reshape(RC, 128, NC2).transpose(1, 0, 2))

    x2 = np.einsum("bf,bf->b", x, x).astype(np.float32)
    xat = np.empty((K1, B), dtype=np.float32)
    xat[:F] = x.T
    xat[F] = 1.0
    xat[F + 1] = x2
    xah, xal = _split_f16(xat)
    xpk = np.concatenate([xah, xal, xah], axis=0)  # [102, B]

    xaug = np.empty((B, F1), dtype=np.float32)
    xaug[:, :F] = x
    xaug[:, F] = 1.0

    in_maps = []
    for ci in range(NCORES):
        sl = slice(ci * BL, (ci + 1) * BL)
        xd = np.empty((KP, NG + 1, GW), dtype=np.float16)
        xd[:, 0, :] = dpack
        xd[:, 1:, :] = xpk[:, sl].reshape(KP, NG, GW)
        xc = (
            xaug[sl].reshape(NG, JG, 128, F1)
            .transpose(2, 0, 1, 3)
            .reshape(128, NG, XCW)
        )
        in_maps.append({
            "xa": np.ascontiguousarray(xd[:, 0:2]),
            "xb": np.ascontiguousarray(xd[:, 2:]),
            "wt": wt,
            "xc": np.ascontiguousarray(xc),
        })
    return in_maps


def assemble_out(results):
    """[128, NG, JG*O] per core -> full [B, O] (b = g*512 + j*128 + p)."""
    outs = []
    for r in results:
        o = np.asarray(r["out"]).reshape(128, NG, JG, O)
        outs.append(o.transpose(1, 2, 0, 3).reshape(BL, O))
    return np.concatenate(outs, axis=0)


def _axon_reset():
    """Recover a wedged NeuronCore (NRT_EXEC_UNIT_UNRECOVERABLE) via the axon
    client's reset entry point.  Best-effort: silently skipped off-axon."""
    try:
        import ctypes
        import time

        import jax

        jax.devices()
        lib = ctypes.CDLL("/opt/axon/libaxon_pjrt.so")
        lib.axon_reset.restype = ctypes.c_int64
        lib.axon_reset()
        time.sleep(2)
    except Exception:
        pass


def kernel(inputs, centers, widths, consequent_w, consequent_b):
    nc = get_program()
    in_maps = make_in_maps(inputs, centers, widths, consequent_w, consequent_b)
    last_err = None
    for attempt in range(3):
        try:
            res = run_bass_kernel_spmd(nc, in_maps, list(range(NCORES))).results
            return assemble_out(res)
        except Exception as e:  # wedged device -> reset + retry
            last_err = e
            _axon_reset()
    raise last_err


# revision 18
# speedup vs baseline: 1.5577x; 1.0216x over previous
"""ANFIS forward kernel for Trainium2 (8 NeuronCores, data-parallel over batch).

Problem shapes (hardcoded): B=16384, R=512 rules, F=32 features, O=8 outputs.

Math (identical to the reference, re-associated for the PE):
  a[r]            = -1 / (2*(|w[r]|+0.1)^2)
  v[r,b]          = a[r] * (x2[b] + c2[r] - 2*cross[b,r])      (= -dist/(2 s^2))
  f[r,b]          = exp(v[r,b])                                 (UNnormalized)
  H[b, o*33+f]    = sum_r f[r,b] * Waug[r,f,o]   (f=32 slot holds the bias)
  H[b, 264]       = sum_r f[r,b]                  (= S[b])
  out[b,o]        = (sum_f xaug[b,f]*H[b,o*33+f]) / (S[b]+1e-8)

Device mapping per core (2048 batch rows, 4 groups of 512):
  MM1 (PE):  psum[128r, 512b] = daug[:, rc]^T @ xaug_g, float32r with K=34.
             On this part f32r streams ~1 col/cycle (same as fp16), so plain
             fp32 beats the old fp16 hi/lo 3-pass trick AND avoids the
             ~190ns PE mode switch between fp16 MM1s and f32r MM2s.
  EXP (ACT): firing tile = exp(psum), fp32 (firing spans e^-13..e^-128;
             16-bit floats would flush/degrade columns whose S ~ 1e-8).
  MM2 (PE):  psum_h[128b, 266] += firing[rc][:, j128]^T @ wt[rc] in float32r
             (~124ns issue rate at N=266 once the PE clock boosts).  Software
             pipelined one group behind MM1 so all four exp tiles of a group
             are ready before its MM2 starts.
  Combine, per j-PAIR over a 2-bank psum tile (GPSIMD cannot read PSUM, so
             everything touching psum is DVE): broadcast mul (xaug repeated
             over o via 0-stride AP) into bf16 scratch + reduce_sum +
             eps-add; one reciprocal per group; final scale on GpSimd
             (SBUF-only) to keep it off DVE.

DMA notes (this part's HWDGE):
  * dma_start issue costs ~650-900ns serialized on the issuing queue;
    first packet lands ~1.8us after issue; a transfer's descriptors spread
    over roughly ceil(ndesc/34) of the 16 DMA engines, so small forced
    descriptors (max_dma_last_dim, in BYTES) buy engine parallelism.
  * The scalar-queue (ACT HWDGE) starts packets earlier than the sync queue,
    so the critical transfer (daug + batch group 0) goes there first.
  * A few junk matmuls on a memset tile keep the PE from idling during the
    input DMA (the PE clock ramps ~2x after ~5us of sustained activity).
  * out = [128, NG, JG*O] f32 partition-major (host inverse-permutes after).

Notes:
  * Built on bacc.Bacc + nc.compile(): generate_event_semaphores legalizes the
    1-wait-per-instruction TRN2 limit, so multi-dep matmuls/DMAs are fine.
  * tensor_tensor_reduce hangs this hardware/runtime combination - do not use.
  * The ~6.5us end-of-kernel semaphore-reset chain is fixed bacc epilogue
    (all 256 sems reset serially across engines) - not reducible from here.
"""

import numpy as np

import concourse.bacc as bacc
import concourse.bass as bass
import concourse.tile as tile
from concourse import mybir
from concourse.bass_utils import run_bass_kernel_spmd

B, R, F, O = 16384, 512, 32, 8
NCORES = 8
BL = B // NCORES           # 2048 batch rows per core
GW = 512                   # batch-group width for MM1 streaming
NG = BL // GW              # 4 groups per core
JG = GW // 128             # 4 b128 tiles per group
K1 = F + 2                 # 34 = x rows + ones row + x2 row
KP = 3 * K1                # 102 = packed contraction dim [h;l;h]
F1 = F + 1                 # 33 = x features + ones (bias slot)
NC2 = O * F1 + 2           # 266 = (o,f+bias) block + S + pad (fp32r even dst)
RC = R // 128              # 4 rule chunks
SCOL = O * F1              # 264: column holding S = sum_r firing
XCW = JG * F1              # 132 xc row elems

F32 = mybir.dt.float32
F16 = mybir.dt.float16
BF16 = mybir.dt.bfloat16
F32R = mybir.dt.float32r
EXP = mybir.ActivationFunctionType.Exp

_PROGRAM = None


def _build_program():
    nc = bacc.Bacc()
    xa_d = nc.declare_dram_parameter("xa", [KP, 2, GW], F16, isOutput=False)
    xb_d = nc.declare_dram_parameter("xb", [KP, NG - 1, GW], F16, isOutput=False)
    wt_d = nc.declare_dram_parameter("wt", [128, RC, NC2], F32R, isOutput=False)
    xc_d = nc.declare_dram_parameter("xc", [128, NG, XCW], F32, isOutput=False)
    out_d = nc.declare_dram_parameter("out", [128, NG, JG * O], F32, isOutput=True)

    with tile.TileContext(nc) as tc:
        with (
            tc.tile_pool(name="one", bufs=1) as one,
            tc.tile_pool(name="ft", bufs=10) as ft_pool,
            tc.tile_pool(name="scr", bufs=2) as scr_pool,
            tc.tile_pool(name="sm", bufs=2) as sm_pool,
            tc.tile_pool(name="ot", bufs=2) as ot_pool,
            tc.tile_pool(name="ps1", bufs=2, space="PSUM") as ps1_pool,
            tc.tile_pool(name="psh", bufs=3, space="PSUM") as psh_pool,
        ):
            # PE warmup while the input DMA is in flight.
            warm = one.tile([128, GW], F16, tag="warm")
            nc.gpsimd.memset(warm[:], 0)
            # slot 0 = dpack, slots 1..4 = batch groups.
            xd_sb = one.tile([KP, NG + 1, GW], F16, tag="xd")
            nc.scalar.dma_start(
                out=xd_sb[:, 0:2], in_=xa_d[:], max_dma_last_dim=1024
            )
            w_sb = one.tile([128, RC, NC2], F32R, tag="wt")
            nc.sync.dma_start(out=w_sb[:], in_=wt_d[:], max_dma_last_dim=1064)
            nc.scalar.dma_start(
                out=xd_sb[:, 2:], in_=xb_d[:], max_dma_last_dim=1024
            )
            xc_sb = one.tile([128, NG, JG, F1], F32, tag="xc")
            nc.scalar.dma_start(
                out=xc_sb[:],
                in_=xc_d[:].rearrange("p g (j f) -> p g j f", f=F1),
                max_dma_last_dim=528,
            )
            wps = ps1_pool.tile([128, GW], F32, tag="ps1")
            for _ in range(8):
                nc.tensor.matmul(
                    wps[:], warm[:, 0:128], warm[:], start=True, stop=True
                )

            fts = [[None] * RC for _ in range(NG)]

            def emit_pair(g, jj, osum_g, sden_g):
                """MM2 + numerator/denominator for j-chunks (2jj, 2jj+1)."""
                psh = psh_pool.tile([128, 2, GW], F32, tag="psh")
                for rc in range(RC):
                    for jp in range(2):
                        j = 2 * jj + jp
                        nc.tensor.matmul(
                            psh[:, jp, 0:NC2],
                            fts[g][rc][:, j * 128:(j + 1) * 128],
                            w_sb[:, rc, :],
                            start=(rc == 0),
                            stop=(rc == RC - 1),
                        )
                # GPSIMD cannot read PSUM, so everything touching psh is DVE.
                # bf16 scratch halves the reduce's input cost.
                xv = xc_sb[:, g, 2 * jj:2 * jj + 2, :]     # [128, 2, F1]
                xb = bass.AP(
                    tensor=xv.tensor,
                    offset=xv.offset,
                    ap=[xv.ap[0], xv.ap[1], [0, O], xv.ap[2]],
                )
                scratch = scr_pool.tile([128, 2, O, F1], BF16, tag="scr")
                nc.vector.tensor_mul(
                    scratch[:],
                    psh[:, :, 0:SCOL].rearrange("p j (o f) -> p j o f", o=O),
                    xb,
                )
                nc.vector.reduce_sum(
                    out=osum_g[:, jj], in_=scratch[:], axis=mybir.AxisListType.X
                )
                nc.vector.tensor_scalar_add(
                    sden_g[:, jj], psh[:, :, SCOL:SCOL + 1], 1e-8
                )

            def emit_mm2_combine(g):
                out_t = ot_pool.tile([128, JG, O], F32, tag="outt")
                osum_g = sm_pool.tile([128, 2, 2, O], F32, tag="osum")
                sden_g = sm_pool.tile([128, 2, 2, 1], F32, tag="sden")
                emit_pair(g, 0, osum_g, sden_g)
                emit_pair(g, 1, osum_g, sden_g)
                rec = sm_pool.tile([128, 2, 2, 1], F32, tag="rec")
                nc.vector.reciprocal(rec[:], sden_g[:])
                ra = rec[:]
                rb = bass.AP(
                    tensor=ra.tensor,
                    offset=ra.offset,
                    ap=[ra.ap[0], ra.ap[1], ra.ap[2], [0, O]],
                )
                # Final scale is SBUF-only, so GpSimd takes it off DVE.
                nc.gpsimd.tensor_mul(
                    out_t[:].rearrange("p (h t) o -> p h t o", h=2), osum_g[:], rb
                )
                nc.sync.dma_start(out=out_d[:, g], in_=out_t[:])

            for g in range(NG):
                for rc in range(RC):
                    ps1 = ps1_pool.tile([128, GW], F32, tag="ps1")
                    nc.tensor.matmul(
                        ps1[:],
                        xd_sb[:, 0, rc * 128:(rc + 1) * 128],
                        xd_sb[:, 1 + g, :],
                        start=True,
                        stop=True,
                    )
                    ft = ft_pool.tile([128, GW], F32R, tag="ft")
                    nc.scalar.activation(ft[:], ps1[:], EXP)
                    fts[g][rc] = ft
                # MM2 one group behind: its 4 exp tiles are already done.
                if g > 0:
                    emit_mm2_combine(g - 1)
            emit_mm2_combine(NG - 1)
    nc.compile()
    return nc


def get_program():
    global _PROGRAM
    if _PROGRAM is None:
        _PROGRAM = _build_program()
    return _PROGRAM


def _split_f16(arr):
    hi = arr.astype(np.float16)
    lo = (arr - hi.astype(np.float32)).astype(np.float16)
    return np.ascontiguousarray(hi), np.ascontiguousarray(lo)


def make_in_maps(inputs, centers, widths, consequent_w, consequent_b):
    x = np.ascontiguousarray(np.asarray(inputs, dtype=np.float32))
    c64 = np.asarray(centers, dtype=np.float64)
    w64 = np.asarray(widths, dtype=np.float64)
    W = np.asarray(consequent_w, dtype=np.float32)
    cb = np.asarray(consequent_b, dtype=np.float32)

    s = np.abs(w64) + 0.1
    a = -1.0 / (2.0 * s * s)                       # [R]

    daug = np.empty((K1, R), dtype=np.float64)
    daug[:F] = (c64 * (-2.0 * a)[:, None]).T       # -2*c[r,f]*a[r]
    daug[F] = (c64 * c64).sum(axis=1) * a          # c2[r]*a[r]  (pairs with ones)
    daug[F + 1] = a                                # a[r]        (pairs with x2)
    dh, dl = _split_f16(daug.astype(np.float32))
    dpack = np.concatenate([dh, dh, dl], axis=0)   # [102, R]

    # Waug columns: col o*33+f = W[r,f,o] (f<32), col o*33+32 = cb[r,o],
    # col 264 = 1.0 (S), col 265 = 0 (pad).
    wt = np.zeros((R, NC2), dtype=np.float32)
    for o in range(O):
        wt[:, o * F1:o * F1 + F] = W[:, :, o]
        wt[:, o * F1 + F] = cb[:, o]
    wt[:, SCOL] = 1.0
    wt = np.ascontiguousarray(wt.reshape(RC, 128, NC2).transpose(1, 0, 2))

    x2 = np.einsum("bf,bf->b", x, x).astype(np.float32)
    xat = np.empty((K1, B), dtype=np.float32)
    xat[:F] = x.T
    xat[F] = 1.0
    xat[F + 1] = x2
    xah, xal = _split_f16(xat)
    xpk = np.concatenate([xah, xal, xah], axis=0)  # [102, B]

    xaug = np.empty((B, F1), dtype=np.float32)
    xaug[:, :F] = x
    xaug[:, F] = 1.0

    in_maps = []
    for ci in range(NCORES):
        sl = slice(ci * BL, (ci + 1) * BL)
        xd = np.empty((KP, NG + 1, GW), dtype=np.float16)
        xd[:, 0, :] = dpack
        xd[:, 1:, :] = xpk[:, sl].reshape(KP, NG, GW)
        xc = (
            xaug[sl].reshape(NG, JG, 128, F1)
            .transpose(2, 0, 1, 3)
            .reshape(128, NG, XCW)
        )
        in_maps.append({
            "xa": np.ascontiguousarray(xd[:, 0:2]),
            "xb": np.ascontiguousarray(xd[:, 2:]),
            "wt": wt,
            "xc": np.ascontiguousarray(xc),
        })
    return in_maps


def assemble_out(results):
    """[128, NG, JG*O] per core -> full [B, O] (b = g*512 + j*128 + p)."""
    outs = []
    for r in results:
        o = np.asarray(r["out"]).reshape(128, NG, JG, O)
        outs.append(o.transpose(1, 2, 0, 3).reshape(BL, O))
    return np.concatenate(outs, axis=0)


def _axon_reset():
    """Recover a wedged NeuronCore (NRT_EXEC_UNIT_UNRECOVERABLE) via the axon
    client's reset entry point.  Best-effort: silently skipped off-axon."""
    try:
        import ctypes
        import time

        import jax

        jax.devices()
        lib = ctypes.CDLL("/opt/axon/libaxon_pjrt.so")
        lib.axon_reset.restype = ctypes.c_int64
        lib.axon_reset()
        time.sleep(2)
    except Exception:
        pass


def kernel(inputs, centers, widths, consequent_w, consequent_b):
    nc = get_program()
    in_maps = make_in_maps(inputs, centers, widths, consequent_w, consequent_b)
    last_err = None
    for attempt in range(3):
        try:
            res = run_bass_kernel_spmd(nc, in_maps, list(range(NCORES))).results
            return assemble_out(res)
        except Exception as e:  # wedged device -> reset + retry
            last_err = e
            _axon_reset()
    raise last_err
